# revision 52
# baseline (speedup 1.0000x reference)
import sys, functools

if "/opt/trn_rl_repo" not in sys.path:
    sys.path.insert(0, "/opt/trn_rl_repo")

import numpy as np
import ml_dtypes

from concourse import bacc
import concourse.bass as bass
import concourse.mybir as mybir
import concourse.tile as tile
from concourse.bass_utils import run_bass_kernel_spmd

BF16 = mybir.dt.bfloat16
F32 = mybir.dt.float32
FP8 = mybir.dt.float8e4
FP8E5 = mybir.dt.float8e5
AF = mybir.ActivationFunctionType
ALU = mybir.AluOpType
AX = mybir.AxisListType
DR = mybir.MatmulPerfMode.DoubleRow

S, D, HD, H, MLPH = 4096, 3072, 128, 24, 9216
NCORES = 8
HL = H // NCORES            # 3 heads per core
FQ = HL * HD                # 384
FM = MLPH // NCORES         # 1152
FMT = FM // 128             # 9 f-tiles of mlp hidden
FAB = 2 * FM                # 2304 (a/b interleaved in 128-col pairs)
FQKV = 3 * FQ               # 1152
NCOL = D // NCORES          # 384 output cols per core
FO = FQ + FM                # 1536 rows of fused output weight
WOT = FO // 128             # 12 contraction tiles of output proj
EPS = 1e-6
SCH = 512                   # s-chunk for projection phase
NSC = S // SCH              # 8
KT = D // 128               # 24 contraction tiles of input proj
NKT = S // 128              # 32 k tiles in attention
QC = 512                    # q-chunk for attention/output phase
NQC = S // QC               # 8
NRS = 8                     # number of ReduceScatter chunks
RSW = S // NRS              # 512 s-columns per RS chunk

LAST_RESULT = None          # test.py introspection


def _rsqrt_dve(nc, pool, u, n, scale, seed, iters):
    """rsqrt(u*scale + EPS) on the vector engine via Newton iteration.

    Values here are tightly concentrated (unit-normal statistics), so a
    constant seed converges in 2-3 iterations; this keeps Sqrt off the
    activation engine, whose table set for sqrt conflicts with silu/exp.
    """
    eng = nc.gpsimd
    w = pool.tile([128, n], F32, tag="nw")
    eng.tensor_scalar(out=w, in0=u, scalar1=scale, scalar2=EPS,
                      op0=ALU.mult, op1=ALU.add)
    y = pool.tile([128, n], F32, tag="ny")
    eng.tensor_scalar(out=y, in0=w, scalar1=-0.5 * seed ** 3,
                      scalar2=1.5 * seed, op0=ALU.mult, op1=ALU.add)
    a = pool.tile([128, n], F32, tag="na")
    for _ in range(iters):
        eng.tensor_mul(out=a, in0=y, in1=y)
        eng.tensor_mul(out=a, in0=a, in1=w)
        eng.tensor_scalar(out=a, in0=a, scalar1=-0.5, scalar2=1.5,
                          op0=ALU.mult, op1=ALU.add)
        eng.tensor_mul(out=y, in0=y, in1=a)
    return y


def _to_bf16(a):
    """Fast round-to-nearest f32 -> bf16."""
    a = np.ascontiguousarray(a, np.float32)
    u = a.view(np.uint32)
    r = ((u >> 16) & 1) + np.uint32(0x7FFF)
    return ((u + r) >> 16).astype(np.uint16).view(ml_dtypes.bfloat16)


@functools.lru_cache(maxsize=1)
def _build():
    nc = bacc.Bacc(
        "TRN2",
        target_bir_lowering=False,
        debug=False,
        enable_asserts=False,
        num_devices=NCORES,
    )
    x = nc.dram_tensor("x", [S, D], BF16, kind="ExternalInput").ap()
    w1q = nc.dram_tensor("w1qkv", [D, FQKV], FP8, kind="ExternalInput").ap()
    w1ab = nc.dram_tensor("w1ab", [D, FAB], BF16, kind="ExternalInput").ap()
    c2q = nc.dram_tensor("c2q", [1, FQKV], BF16, kind="ExternalInput").ap()
    c2ab = nc.dram_tensor("c2ab", [128, 2 * FMT], F32, kind="ExternalInput").ap()
    cosb = nc.dram_tensor("cosb", [S, HD], BF16, kind="ExternalInput").ap()
    sinb = nc.dram_tensor("sinb", [S, HD], BF16, kind="ExternalInput").ap()
    qwb = nc.dram_tensor("qwb", [128, HD], F32, kind="ExternalInput").ap()
    kwb = nc.dram_tensor("kwb", [128, HD], F32, kind="ExternalInput").ap()
    wout = nc.dram_tensor("wout", [FO, D], BF16, kind="ExternalInput").ap()
    resT = nc.dram_tensor("resT", [NCOL, S], F32, kind="ExternalInput").ap()
    out_t = nc.dram_tensor("out", [NCOL, S], F32, kind="ExternalOutput").ap()

    rg = [list(range(NCORES))]

    with tile.TileContext(nc) as tc:
        with (
            tc.tile_pool(name="const", bufs=1) as const,
            tc.tile_pool(name="attk", bufs=1) as attk,
            tc.tile_pool(name="dram", bufs=1, space="DRAM") as dram,
        ):
            kT_sb = attk.tile([128, HL, S], BF16)
            v_sb8 = attk.tile([128, NKT, FQ], FP8)
            ones_p = const.tile([1, 128], BF16)
            nc.vector.memset(ones_p, 1.0)
            ones8 = const.tile([128, 2, 128], FP8E5)
            nc.vector.memset(ones8, 1.0)
            eps_sb = const.tile([128, 1], F32)
            nc.vector.memset(eps_sb, EPS)
            bm2_sb = const.tile([128, 1], F32)
            nc.vector.memset(bm2_sb, -2.0)
            qwb_sb = const.tile([128, HD], F32)
            nc.sync.dma_start(qwb_sb, qwb)
            kwb_sb = const.tile([128, HD], F32)
            nc.sync.dma_start(kwb_sb, kwb)
            c2q_sb = const.tile([1, FQKV], BF16)
            nc.sync.dma_start(c2q_sb, c2q)
            c2ab_sb = const.tile([128, 2 * FMT], F32)
            nc.sync.dma_start(c2ab_sb, c2ab)

            tti_d = dram.tile([S, D], BF16)     # normalized x, s-major
            qr_d = dram.tile([S, FQ], BF16)     # rope(q), s-major
            kr_d = dram.tile([S, FQ], BF16)
            v_d8 = dram.tile([S, FQ], FP8)
            m_f = dram.tile([FM, S], BF16)      # swiglu output, f-major
            # RS chunks: one per q-chunk, the last split in half so the
            # final collective + epilogue tail is shorter
            chunks = [(i * RSW, RSW) for i in range(NRS - 1)]
            chunks += [(S - RSW, RSW // 2), (S - RSW // 2, RSW // 2)]
            pall = [dram.tile([D, w], BF16, tag=f"pall{i}", name=f"pall{i}")
                    for i, (c0, w) in enumerate(chunks)]
            rs_out = [dram.tile([NCOL, w], BF16, tag=f"rso{i}", name=f"rso{i}")
                      for i, (c0, w) in enumerate(chunks)]

            # ---------------- Phase P: LN + transpose + QKV/MLP projection ----
            with (
                tc.tile_pool(name="w1qp", bufs=1) as w1qp,
                tc.tile_pool(name="xp", bufs=2) as xp,
                tc.tile_pool(name="tp", bufs=2) as tp,
                tc.tile_pool(name="ttp", bufs=2) as ttp,
                tc.tile_pool(name="w1s", bufs=3) as w1s,
                tc.tile_pool(name="smal", bufs=12) as smal,
                tc.tile_pool(name="stg", bufs=2) as stg,
                tc.tile_pool(name="abp", bufs=3) as abp,
                tc.tile_pool(name="psab", bufs=2, space="PSUM") as psab,
                tc.tile_pool(name="psq", bufs=4, space="PSUM") as psq,
            ):
                # resident QKV weight: [dm_part, kt, 1152]; loaded in 4 pieces
                # interleaved with the first LN tiles to not block the x loads
                w1q_sb = w1qp.tile([128, KT, FQKV], FP8)
                w1q_r = w1q.rearrange("(kt p) f -> p kt f", p=128)

                for sc in range(NSC):
                    tT = ttp.tile([128, KT, SCH], BF16, tag="tT")
                    tT8 = ttp.tile([128, KT, SCH], FP8, tag="tT8", bufs=1)
                    for ss in range(4):
                        s0 = sc * SCH + ss * 128
                        xt = xp.tile([128, D], BF16, tag="x")
                        nc.sync.dma_start(xt, x[s0 : s0 + 128, :])
                        if sc == 0 and ss < 4:
                            k6 = KT // 4
                            nc.sync.dma_start(
                                w1q_sb[:, ss * k6 : (ss + 1) * k6, :],
                                w1q_r[:, ss * k6 : (ss + 1) * k6, :])
                        s1 = smal.tile([128, 1], F32, tag="s1")
                        nc.vector.reduce_sum(s1, xt, axis=AX.X)
                        nmu = smal.tile([128, 1], F32, tag="nmu")
                        nc.scalar.mul(nmu, s1, -1.0 / D)
                        sqs = tp.tile([128, D], BF16, tag="sq", bufs=1)
                        v2 = smal.tile([128, 1], F32, tag="v2")
                        nc.scalar.activation(sqs, xt, AF.Square, bias=nmu, scale=1.0,
                                             accum_out=v2)
                        std = smal.tile([128, 1], F32, tag="std")
                        nc.scalar.activation(std, v2, AF.Sqrt, bias=eps_sb,
                                             scale=1.0 / D)
                        rstd = smal.tile([128, 1], F32, tag="rstd")
                        nc.vector.reciprocal(rstd, std)
                        nmr = smal.tile([128, 1], F32, tag="nmr")
                        nc.vector.tensor_mul(out=nmr, in0=nmu, in1=rstd)
                        tti = tp.tile([128, D], BF16, tag="t")
                        nc.scalar.activation(tti, xt, AF.Identity, bias=nmr, scale=rstd)
                        # round-trip through DRAM + XBAR dma transpose -> tT
                        nc.sync.dma_start(tti_d[s0 : s0 + 128, :], tti)
                        nc.sync.dma_start_transpose(
                            tT[:, :, ss * 128 : (ss + 1) * 128],
                            tti_d[s0 : s0 + 128, :])
                        nc.vector.tensor_copy(
                            out=tT8[:, :, ss * 128 : (ss + 1) * 128],
                            in_=tT[:, :, ss * 128 : (ss + 1) * 128])

                    # --- QKV pass (s-major): one psum bank per j ---
                    for ss in range(4):
                        s0 = sc * SCH + ss * 128
                        sl = ss * 128
                        cos_t = stg.tile([128, HD], BF16, tag="cos")
                        nc.sync.dma_start(cos_t, cosb[s0 : s0 + 128, :])
                        sin_t = stg.tile([128, HD], BF16, tag="sin")
                        nc.sync.dma_start(sin_t, sinb[s0 : s0 + 128, :])
                        s2_ = sin_t.rearrange("p (x two) -> p x two", two=2)
                        pqs = [psq.tile([128, 512], F32, tag="pqkv",
                                        name=f"pq{j}") for j in range(3)]
                        for kt2 in range(KT // 2):
                            for j in range(3):
                                nc.tensor.matmul(
                                    pqs[j][:, :FQ],
                                    tT8[:, 2 * kt2 : 2 * kt2 + 2, sl : sl + 128],
                                    w1q_sb[:, 2 * kt2 : 2 * kt2 + 2,
                                           j * FQ : (j + 1) * FQ],
                                    start=(kt2 == 0), stop=False,
                                    perf_mode=DR)
                        for j in range(3):
                            pq = pqs[j]
                            nc.tensor.matmul(
                                pq[:, :FQ], ones_p,
                                c2q_sb[:, j * FQ : (j + 1) * FQ],
                                start=False, stop=True)
                            if j == 2:
                                vstg = stg.tile([128, FQ], FP8, tag="vst", bufs=2)
                                nc.scalar.copy(vstg, pq[:, :FQ])
                                nc.sync.dma_start(v_d8[s0 : s0 + 128, :], vstg)
                                continue
                            wb = qwb_sb if j == 0 else kwb_sb
                            qn = stg.tile([128, FQ], BF16, tag=f"qn{j}")
                            qrr = stg.tile([128, FQ], BF16, tag=f"qr{j}")
                            tmp = stg.tile([128, FQ], BF16, tag=f"tm{j}")
                            # batch the per-head RMS stats so one Sqrt/recip
                            # serves all heads (fewer ACT table switches)
                            ssq3 = smal.tile([128, HL], F32, tag="ssq3")
                            for hh in range(HL):
                                sq2 = stg.tile([128, HD], F32, tag="sq2")
                                nc.scalar.activation(
                                    sq2, pq[:, hh * HD : (hh + 1) * HD],
                                    AF.Square, accum_out=ssq3[:, hh : hh + 1])
                            sstd = smal.tile([128, HL], F32, tag="sstd")
                            nc.scalar.activation(sstd, ssq3, AF.Sqrt,
                                                 bias=eps_sb, scale=1.0 / HD)
                            rst3 = smal.tile([128, HL], F32, tag="rst3")
                            nc.vector.reciprocal(rst3, sstd)
                            # rope runs on the (otherwise idle) gpsimd engine:
                            # its outputs are consumed far away in phase A, and
                            # keeping these ~40 micro-ops off the vector FIFO
                            # unblocks the next chunk's LN/cast chain
                            for hh in range(HL):
                                blk = pq[:, hh * HD : (hh + 1) * HD]
                                qnb = qn[:, hh * HD : (hh + 1) * HD]
                                nc.vector.scalar_tensor_tensor(
                                    qnb, blk, rst3[:, hh : hh + 1], wb,
                                    ALU.mult, ALU.mult)
                                q3 = qnb.rearrange("p (x two) -> p x two", two=2)
                                t3 = tmp[:, hh * HD : (hh + 1) * HD].rearrange(
                                    "p (x two) -> p x two", two=2)
                                nc.gpsimd.tensor_mul(out=t3[:, :, 0], in0=q3[:, :, 1],
                                                     in1=s2_[:, :, 0])
                                nc.gpsimd.tensor_mul(out=t3[:, :, 1], in0=q3[:, :, 0],
                                                     in1=s2_[:, :, 1])
                                nc.gpsimd.tensor_mul(
                                    out=qrr[:, hh * HD : (hh + 1) * HD],
                                    in0=qnb, in1=cos_t)
                            nc.gpsimd.tensor_add(out=qrr, in0=qrr, in1=tmp)
                            dst = qr_d if j == 0 else kr_d
                            nc.sync.dma_start(dst[s0 : s0 + 128, :], qrr)

                    # k/v for this s-chunk are final: transpose k into the
                    # resident attention-K tile and pull v in as fp8 now, so
                    # the phase transition has no big serial transfer left
                    nc.sync.dma_start_transpose(
                        kT_sb[:, :, sc * SCH : (sc + 1) * SCH],
                        kr_d[sc * SCH : (sc + 1) * SCH, :])
                    nc.sync.dma_start(
                        v_sb8[:, sc * 4 : (sc + 1) * 4, :],
                        v_d8.rearrange("(t p) f -> p t f", p=128)[
                            :, sc * 4 : (sc + 1) * 4, :])

                    # --- a/b (f-major) + SwiGLU ---
                    for fb in range(FMT):
                        pa = psab.tile([128, 2, SCH], F32, tag="pab")
                        for kg in range(KT // 8):
                            wt = w1s.tile([128, 8, 256], BF16, tag="w1ab")
                            nc.sync.dma_start(
                                wt, w1ab.rearrange("(kt p) f -> p kt f", p=128)[
                                    :, kg * 8 : (kg + 1) * 8,
                                    fb * 256 : (fb + 1) * 256])
                            for k8 in range(8):
                                kt = kg * 8 + k8
                                for f2 in range(2):
                                    nc.tensor.matmul(
                                        pa[:, f2, :],
                                        wt[:, k8, f2 * 128 : (f2 + 1) * 128],
                                        tT[:, kt, :],
                                        start=(kt == 0), stop=(kt == KT - 1))
                        a_sb = abp.tile([128, SCH], BF16, tag="asb")
                        nc.scalar.activation(a_sb, pa[:, 0, :], AF.Silu,
                                             bias=c2ab_sb[:, 2 * fb : 2 * fb + 1])
                        m_sb = abp.tile([128, SCH], BF16, tag="msb")
                        nc.vector.scalar_tensor_tensor(
                            m_sb, pa[:, 1, :], c2ab_sb[:, 2 * fb + 1 : 2 * fb + 2],
                            a_sb, ALU.add, ALU.mult)
                        nc.sync.dma_start(
                            m_f[fb * 128 : (fb + 1) * 128,
                                sc * SCH : (sc + 1) * SCH],
                            m_sb)

            # ---------------- Phase A+O: attention + output proj + RS ---------
            with (
                tc.tile_pool(name="wo", bufs=1) as wo,
                tc.tile_pool(name="qtp", bufs=2) as qtp,
                tc.tile_pool(name="mop", bufs=3) as mop,
                tc.tile_pool(name="ptp", bufs=4) as ptp,
                tc.tile_pool(name="atts", bufs=2) as atts,
                tc.tile_pool(name="pop", bufs=3) as pop,
                tc.tile_pool(name="eop", bufs=1) as eop,
                tc.tile_pool(name="psS", bufs=2, space="PSUM") as psS,
                tc.tile_pool(name="psDV", bufs=1, space="PSUM") as psDV,
                tc.tile_pool(name="psO", bufs=2, space="PSUM") as psO,
            ):
                qts, mts = {}, {}

                def load_qc(qc):
                    q0 = qc * QC
                    qt = qtp.tile([128, HL, QC], BF16, tag="qt", name="qt")
                    nc.sync.dma_start_transpose(qt, qr_d[q0 : q0 + QC, :])
                    mt = mop.tile([128, FMT, QC], BF16, tag="mt", name="mt")
                    nc.sync.dma_start(
                        mt, m_f.rearrange("(t p) s -> p t s", p=128)[
                            :, :, q0 : q0 + QC])
                    qts[qc], mts[qc] = qt, mt

                load_qc(0)
                # load in 4 column chunks so outproj's first dt tiles don't
                # wait for the whole 9.4MB transfer (which itself can't start
                # until phase P's SBUF frees)
                wo_sb = wo.tile([128, WOT, D], BF16)
                wo_r = wout.rearrange("(kt p) n -> p kt n", p=128)
                for i in range(4):
                    nc.sync.dma_start(
                        wo_sb[:, :, i * (D // 4) : (i + 1) * (D // 4)],
                        wo_r[:, :, i * (D // 4) : (i + 1) * (D // 4)])

                def epilogue(ri):
                    # keep epilogue traffic on the otherwise-idle gpsimd
                    # queue: the scheduler hoists it right after its RS, and
                    # on sync/vector it head-of-line blocks the next chunk's
                    # loads. The last chunk runs after all compute, so its
                    # add can use the (then idle) vector engine.
                    c0, w = chunks[ri]
                    rsb = eop.tile([128, HL, w], BF16, tag="rsb", name="rsb")
                    nc.gpsimd.dma_start(
                        rsb, rs_out[ri].rearrange("(t p) s -> p t s", p=128))
                    rt = eop.tile([128, HL, w], F32, tag="rt", name="rt")
                    nc.gpsimd.dma_start(
                        rt, resT.rearrange("(t p) s -> p t s", p=128)[
                            :, :, c0 : c0 + w])
                    ot = eop.tile([128, HL, w], F32, tag="ot", name="ot")
                    if ri >= len(chunks) - 1:
                        nc.vector.tensor_add(out=ot, in0=rt, in1=rsb)
                        nc.sync.dma_start(
                            out_t.rearrange("(t p) s -> p t s", p=128)[
                                :, :, c0 : c0 + w],
                            ot)
                    else:
                        nc.gpsimd.tensor_add(out=ot, in0=rt, in1=rsb)
                        nc.gpsimd.dma_start(
                            out_t.rearrange("(t p) s -> p t s", p=128)[
                                :, :, c0 : c0 + w],
                            ot)

                def outproj(ci, oaos, omt, moff, w):
                    # output projection partial: [D, w] = woutT @ [attn; mlp]
                    for dt in range(KT):
                        po = psO.tile([128, QC], F32, tag="po", name="po")
                        for t in range(WOT):
                            rhs = (oaos[t][:, :w] if t < HL
                                   else omt[:, t - HL, moff : moff + w])
                            nc.tensor.matmul(
                                po[:, :w],
                                wo_sb[:, t, dt * 128 : (dt + 1) * 128], rhs,
                                start=(t == 0), stop=(t == WOT - 1))
                        pout = pop.tile([128, QC], BF16, tag="pout",
                                        name="pout")
                        if dt % 2 == 0:
                            nc.scalar.copy(pout[:, :w], po[:, :w])
                        else:
                            nc.vector.tensor_copy(out=pout[:, :w],
                                                  in_=po[:, :w])
                        nc.sync.dma_start(
                            pall[ci][dt * 128 : (dt + 1) * 128, :],
                            pout[:, :w])
                    nc.gpsimd.collective_compute(
                        "ReduceScatter", ALU.add, replica_groups=rg,
                        ins=[pall[ci].opt()], outs=[rs_out[ci].opt()])

                # one work item per RS chunk: the last q-chunk is processed as
                # two 256-col halves so outproj(7a) overlaps attention(7b) and
                # the serial tail shrinks to half an attention + outproj
                prev = qt = mt = None
                for ci, (c0, cw) in enumerate(chunks):
                    qc, off = c0 // QC, c0 % QC
                    if off == 0:
                        if qc + 1 < NQC:
                            load_qc(qc + 1)
                        qt, mt = qts.pop(qc), mts.pop(qc)
                    aos = []
                    for h in range(HL):
                        pdv = psDV.tile([128, 2, QC], F32, tag="pdv", name="pdv")
                        pend = []

                        def drain(pdv=pdv, h=h, w=cw):
                            pt, pr = pend.pop(0)
                            nc.tensor.matmul(
                                pdv[:, 0, :w], ones8, pt[:, :, :w],
                                start=(pr == 0), stop=(pr == NKT // 2 - 1),
                                perf_mode=DR)
                            nc.tensor.matmul(
                                pdv[:, 1, :w],
                                v_sb8[:, 2 * pr : 2 * pr + 2,
                                      h * HD : (h + 1) * HD],
                                pt[:, :, :w],
                                start=(pr == 0), stop=(pr == NKT // 2 - 1),
                                perf_mode=DR)

                        for k2 in range(NKT // 2):
                            pss = psS.tile([128, 2, QC], F32, tag="pss", name="pss")
                            for kk in range(2):
                                ki = k2 * 2 + kk
                                nc.tensor.matmul(
                                    pss[:, kk, :cw],
                                    kT_sb[:, h, ki * 128 : (ki + 1) * 128],
                                    qt[:, h, off : off + cw],
                                    start=True, stop=True)
                            if len(pend) == 2:
                                drain()
                            pt = ptp.tile([128, 2, QC], FP8E5, tag="pt", name="pt")
                            nc.scalar.activation(pt[:, :, :cw], pss[:, :, :cw],
                                                 AF.Exp, bias=bm2_sb)
                            pend.append((pt, k2))
                        while pend:
                            drain()
                        invd = atts.tile([128, QC], F32, tag="invd", name="invd")
                        nc.vector.reciprocal(invd[:, :cw], pdv[:, 0, :cw])
                        ao = atts.tile([128, QC], BF16, tag=f"ao{h}", name=f"ao{h}")
                        nc.vector.tensor_mul(out=ao[:, :cw], in0=pdv[:, 1, :cw],
                                             in1=invd[:, :cw])
                        aos.append(ao)
                    # pipeline the output projection one chunk behind the
                    # attention math: during this chunk's ACT-bound softmax
                    # stretches the PE fills with ready outproj matmuls
                    if prev is not None:
                        outproj(*prev)
                    prev = (ci, aos, mt, off, cw)
                outproj(*prev)
                for ri in range(len(chunks)):
                    epilogue(ri)

    nc.finalize()
    return nc


def _prep(inputs):
    hs = np.asarray(inputs["hidden_states"], np.float32).reshape(S, D)
    temb = np.asarray(inputs["temb_mod"], np.float32).reshape(3 * D)
    shift, scale, gate = temb[:D], temb[D : 2 * D], temb[2 * D :]
    cos = np.asarray(inputs["rotary_cos"], np.float32)
    sin = np.asarray(inputs["rotary_sin"], np.float32)
    w1 = np.asarray(inputs["w_qkv_mlp"], np.float32)
    wa = np.asarray(inputs["w_out_attn"], np.float32)
    wm = np.asarray(inputs["w_out_mlp"], np.float32)
    nqw = np.asarray(inputs["norm_q_w"], np.float32)
    nkw = np.asarray(inputs["norm_k_w"], np.float32)

    sgn = np.ones(HD, np.float32)
    sgn[0::2] = -1.0
    xb = _to_bf16(hs)
    cosb = _to_bf16(cos)
    sinb = _to_bf16(sin * sgn)
    alpha = float(HD) ** -0.25
    qwb = np.tile((nqw * alpha)[None, :], (128, 1)).astype(np.float32)
    kwb = np.tile((nkw * alpha)[None, :], (128, 1)).astype(np.float32)
    onep = (1.0 + scale)[:, None]

    in_maps = []
    for c in range(NCORES):
        q0, k0, v0 = c * FQ, D + c * FQ, 2 * D + c * FQ
        a0, b0 = 3 * D + c * FM, 3 * D + MLPH + c * FM
        w1qkv = np.concatenate(
            [w1[:, q0 : q0 + FQ], w1[:, k0 : k0 + FQ], w1[:, v0 : v0 + FQ]], axis=1)
        a_c = w1[:, a0 : a0 + FM].reshape(D, FMT, 128)
        b_c = w1[:, b0 : b0 + FM].reshape(D, FMT, 128)
        w1ab_c = np.stack([a_c, b_c], axis=2).reshape(D, FAB)
        c2q_c = (shift @ w1qkv)[None, :]
        c2ab_c = (shift @ w1ab_c).reshape(2 * FMT, 128).T
        n0 = c * NCOL
        wout_c = np.concatenate(
            [wa[c * FQ : (c + 1) * FQ, :], wm[c * FM : (c + 1) * FM, :]],
            axis=0) * gate[None, :]
        in_maps.append(dict(
            x=xb,
            w1qkv=np.asarray(w1qkv * onep, np.float32).astype(ml_dtypes.float8_e4m3fn),
            w1ab=_to_bf16(w1ab_c * onep),
            c2q=_to_bf16(c2q_c),
            c2ab=np.ascontiguousarray(c2ab_c, np.float32),
            cosb=cosb, sinb=sinb, qwb=qwb, kwb=kwb,
            wout=_to_bf16(wout_c),
            resT=np.ascontiguousarray(hs[:, n0 : n0 + NCOL].T),
        ))
    return in_maps


def kernel(**inputs):
    global LAST_RESULT
    nc = _build()
    in_maps = _prep(inputs)
    r = run_bass_kernel_spmd(nc, in_maps, core_ids=list(range(NCORES)))
    LAST_RESULT = r
    full = np.concatenate([m["out"].T for m in r.results], axis=1)
    return full.reshape(1, S, D).astype(np.float32)
